# revision 12
# baseline (speedup 1.0000x reference)
"""ChebConv2D (K1=K2=3) Trainium2 Bass kernel.

Data-parallel over batch (B=8) across 8 NeuronCores; per core the whole
per-batch computation runs on-chip.

Math (per batch, x: [N, N, C], N=200, C=32, OUT=64):
    out = U_0 + R_L(U_1) + R_{L^2}(U_2) + bias
    U_j = sum_i (A^i x) @ W'_ij^T      (Chebyshev folded into W' on host)

v4: no DRAM scratch. i=0 power comes from a host-transposed copy of x
DMA'd straight into TT; S1 computes only the L and L^2 powers and
scatters them into TT via SBUF->SBUF DMAs (4 chunks per DMA). Output is
stored [n2, n1, o] fp16 with 5 blocks per store DMA; the host transposes
back and upcasts.
"""

import numpy as np

import concourse.bass as bass
import concourse.mybir as mybir
from concourse import bacc
import concourse.tile as tile
from concourse import bass_utils

N = 200
C = 32
OUT = 64
B = 8
NC_HALF = 100
BLK = 8
NBLK = N // BLK
F32 = mybir.dt.float32
F16 = mybir.dt.float16
MIXN = 192
SUP = 4           # S1 chunks per scatter super-chunk
NSUP = 13         # ceil(50 / 4); last super has 2 chunks
OGRP = 5          # output blocks per store DMA


def build_program():
    nc = bacc.Bacc("TRN2")

    x_d = nc.dram_tensor("x", [N, N * C], F16, kind="ExternalInput")
    xt_d = nc.dram_tensor("xt", [C, N * N], F16, kind="ExternalInput")
    g_d = nc.dram_tensor("g", [N, 2 * N], F16, kind="ExternalInput")
    ws_d = nc.dram_tensor("ws", [C * 3 + 1, MIXN], F16, kind="ExternalInput")
    lt1_d = nc.dram_tensor("lt1", [N, N], F16, kind="ExternalInput")
    lt2_d = nc.dram_tensor("lt2", [N, N], F16, kind="ExternalInput")
    ones_d = nc.dram_tensor("ones", [1, N * N], F16, kind="ExternalInput")
    # out is [n2, n1, o]; host transposes back
    out_d = nc.dram_tensor("out", [N, N, OUT], F16, kind="ExternalOutput")

    with tile.TileContext(nc) as tc:
        with (
            tc.tile_pool(name="const", bufs=1) as constp,
            tc.tile_pool(name="tt", bufs=1) as ttp,
            tc.tile_pool(name="u0", bufs=1) as u0p,
        ):
            g_t = []
            lt_t = {}
            for t in range(2):
                g = constp.tile([NC_HALF, 2 * N], F16, tag=f"g{t}")
                nc.sync.dma_start(g[:], g_d[t * NC_HALF:(t + 1) * NC_HALF, :])
                g_t.append(g)
                for j in (1, 2):
                    lt = constp.tile([NC_HALF, N], F16, tag=f"lt{j}{t}")
                    src = lt1_d if j == 1 else lt2_d
                    nc.sync.dma_start(lt[:], src[t * NC_HALF:(t + 1) * NC_HALF, :])
                    lt_t[(j, t)] = lt
            ws = constp.tile([C * 3 + 1, MIXN], F16, tag="ws")
            nc.sync.dma_start(ws[:], ws_d[:, :])

            TT = ttp.tile([C * 3 + 1, N * N], F16, tag="TT")
            # i=0 power = x itself, in transposed layout [c, n2, n1]
            nc.sync.dma_start(TT[0:C, :], xt_d[:, :])
            nc.sync.dma_start(TT[96:97, :], ones_d[:, :])
            TT3 = TT[:].rearrange("p (a b) -> p a b", b=N)

            # U half 0 (even/first-half m) for all n1: [m 0..99, n1*192 + (j,o)]
            UC0 = u0p.tile([NC_HALF, N * MIXN], F16, tag="UC0")

            XCH = 5
            with (
                tc.tile_pool(name="xa", bufs=4) as xap,
                tc.tile_pool(name="sg", bufs=2) as sgp,
                tc.tile_pool(name="uc", bufs=4) as ucp,
                tc.tile_pool(name="ob", bufs=2) as obp,
                tc.tile_pool(name="psU", bufs=2, space="PSUM") as psup,
            ):
                # ---- S1 + scatter per super-chunk ----
                psap_cm = tc.tile_pool(name="psA", bufs=3, space="PSUM")
                psap = psap_cm.__enter__()
                xt_big = [None, None]

                xq = {}

                def x_load(m):
                    for t in range(2):
                        xm = xap.tile([NC_HALF, XCH * 128], F16,
                                      tag=f"xm{t}", name=f"xm{t}_{m}")
                        nc.scalar.dma_start(
                            xm[:], x_d[t * NC_HALF:(t + 1) * NC_HALF,
                                       m * 128:(m + XCH) * 128])
                        xq[(t, m)] = xm

                def s1_super(s):
                    k0 = s * SUP
                    nk = min(SUP, 50 - k0)
                    # sc free layout: (i 2, k SUP, e N)
                    sc = sgp.tile([128, SUP * 2 * N], F16, tag="sc",
                                  name=f"sc_{s}")
                    sc4 = sc[:].rearrange("p (i k e) -> p i k e", i=2, e=N)
                    for k in range(nk):
                        m = k0 + k
                        if m % XCH == 0:
                            if (0, m) not in xq:
                                x_load(m)
                            if m + XCH < 50 and (0, m + XCH) not in xq:
                                x_load(m + XCH)
                            for t in range(2):
                                xt_big[t] = xq[(t, m)]
                        mm = m % XCH
                        psa = psap.tile([128, 2 * N], F32, tag="psa")
                        for t in range(2):
                            lhsT = xt_big[t][:, mm * 128:(mm + 1) * 128]
                            nc.tensor.matmul(psa[:], lhsT, g_t[t][:, :],
                                             start=(t == 0), stop=(t == 1))
                        dst = sc4[:, :, k, :]
                        src = psa[:].rearrange("p (i e) -> p i e", e=N)
                        if k % 2 == 0:
                            nc.vector.tensor_copy(dst, src)
                        else:
                            nc.scalar.copy(dst, src)
                    # scatter: sc[(c r), (i k e)] -> TT[(i c), mem 16s+4r+k, e]
                    # (x cols are host-reordered so psum partitions are c-major;
                    #  TT node dim is in permuted memory order; host compensates)
                    for i in range(2):
                        src = sc[:, i * SUP * N:i * SUP * N + nk * N]
                        dst = TT3[(1 + i) * C:(2 + i) * C,
                                  4 * k0:4 * (k0 + nk), :]
                        nc.sync.dma_start(dst, src)

                def s2h0_pair(p2):
                    psu = psup.tile([NC_HALF, 2 * MIXN], F32, tag="psu",
                                    name=f"psu0_{p2}")
                    for q in range(2):
                        n1 = p2 * 2 + q
                        lhsT = TT3[0:97, 0:NC_HALF, n1:n1 + 1]
                        nc.tensor.matmul(psu[:, q * MIXN:(q + 1) * MIXN],
                                         lhsT, ws[:], start=True, stop=True)
                    dst = UC0[:].rearrange("p (n f) -> p n f", f=MIXN)[
                        :, p2 * 2:p2 * 2 + 2, :]
                    psu3 = psu[:].rearrange("p (q f) -> p q f", f=MIXN)
                    if p2 % 2 == 0:
                        nc.vector.tensor_copy(dst, psu3)
                    else:
                        nc.scalar.copy(dst, psu3)

                # S1 supers 0..6 cover n2 0..111 (>= half 0), then
                # interleave remaining supers with S2 h=0
                for s in range(7):
                    s1_super(s)
                p2done = 0
                for s in range(7, NSUP):
                    s1_super(s)
                    target = (s - 6) * 14
                    while p2done < min(target, 100):
                        s2h0_pair(p2done)
                        p2done += 1
                while p2done < 100:
                    s2h0_pair(p2done)
                    p2done += 1
                psap_cm.__exit__(None, None, None)

                # ---- S2 h=1 + S3 per block ----
                UC03 = UC0[:].rearrange("p (n f) -> p n f", f=MIXN)
                psop = ctx_psop = tc.tile_pool(name="psO", bufs=2, space="PSUM")
                psop = psop.__enter__()
                obig = {}
                for blk in range(NBLK):
                    uc1 = ucp.tile([NC_HALF, BLK * MIXN], F16, tag="uc1",
                                   name=f"uc1_{blk}")
                    for bi2 in range(BLK // 2):
                        psu = psup.tile([NC_HALF, 2 * MIXN], F32, tag="psu")
                        for q in range(2):
                            n1 = blk * BLK + bi2 * 2 + q
                            lhsT = TT3[0:97, NC_HALF:N, n1:n1 + 1]
                            nc.tensor.matmul(psu[:, q * MIXN:(q + 1) * MIXN],
                                             lhsT, ws[:], start=True, stop=True)
                        dst = uc1[:].rearrange("p (n f) -> p n f", f=MIXN)[
                            :, bi2 * 2:bi2 * 2 + 2, :]
                        psu3 = psu[:].rearrange("p (q f) -> p q f", f=MIXN)
                        if bi2 % 2 == 0:
                            nc.vector.tensor_copy(dst, psu3)
                        else:
                            nc.scalar.copy(dst, psu3)
                    uc13 = uc1[:].rearrange("p (n f) -> p n f", f=MIXN)
                    gi = blk // OGRP
                    go = blk % OGRP
                    if go == 0:
                        for m2 in range(2):
                            obig[(gi, m2)] = obp.tile(
                                [NC_HALF, OGRP * BLK * OUT], F16, tag=f"ob{m2}",
                                name=f"ob{m2}_{gi}")
                    for m2 in range(2):
                        pso = psop.tile([NC_HALF, BLK * OUT], F32, tag="pso")
                        k = 0
                        for j in (1, 2):
                            for h in range(2):
                                lhsT = lt_t[(j, h)][:, m2 * NC_HALF:(m2 + 1) * NC_HALF]
                                u3 = UC03 if h == 0 else uc13
                                if h == 0:
                                    rhs = u3[:, blk * BLK:(blk + 1) * BLK,
                                             j * OUT:(j + 1) * OUT]
                                else:
                                    rhs = u3[:, :, j * OUT:(j + 1) * OUT]
                                nc.tensor.matmul(pso[:], lhsT, rhs,
                                                 start=(k == 0), stop=(k == 3))
                                k += 1
                        pso3 = pso[:].rearrange("p (n o) -> p n o", o=OUT)
                        u0s = (UC03 if m2 == 0 else uc13)
                        if m2 == 0:
                            u0 = u0s[:, blk * BLK:(blk + 1) * BLK, 0:OUT]
                        else:
                            u0 = u0s[:, :, 0:OUT]
                        ob3 = obig[(gi, m2)][:].rearrange(
                            "p (g n o) -> p g n o", g=OGRP, o=OUT)[:, go]
                        nc.vector.tensor_add(ob3, pso3, u0)
                        if go == OGRP - 1:
                            dst = out_d[m2 * NC_HALF:(m2 + 1) * NC_HALF,
                                        gi * OGRP * BLK:(gi + 1) * OGRP * BLK, :]
                            src = obig[(gi, m2)][:].rearrange(
                                "p (n o) -> p n o", o=OUT)
                            nc.gpsimd.dma_start(dst, src)
                ctx_psop.__exit__(None, None, None)
    nc.compile()
    return nc


def _perm():
    # TT node-dim memory order: within a full super (4 chunks of 4 rows),
    # mem row 16s+4r+k holds logical n2 16s+4k+r; tail super (2 chunks):
    # mem 192+2r+k holds logical 192+4k+r.
    P = np.zeros(N, np.int64)
    for s in range(12):
        base = 16 * s
        for a in range(4):
            for b in range(4):
                P[base + 4 * a + b] = base + 4 * b + a
    for r in range(4):
        for k in range(2):
            P[192 + 2 * r + k] = 192 + 4 * k + r
    return P


PERM = _perm()


def _host_inputs(adj, weight, bias):
    adj = np.asarray(adj, np.float64)
    weight = np.asarray(weight, np.float64)
    bias = np.asarray(bias, np.float64)
    n = adj.shape[0]
    A = adj * (1.0 - np.eye(n))
    d0 = A.sum(0) ** -0.5
    d1 = A.sum(1) ** -0.5
    d0[np.isinf(d0)] = 0.0
    d1[np.isinf(d1)] = 0.0
    L = d0[:, None] * A * d1[None, :]
    L2 = L @ L

    p = np.array([[1.0, 0, 0], [0, 1.0, 0], [-1.0, 0, 2.0]])
    W = weight.reshape(OUT, 3, 3, C)
    Wp = np.einsum("ai,bj,oabc->ijoc", p, p, W)

    G = np.concatenate([L, L2], axis=1)
    WS = np.zeros((3 * C + 1, MIXN))
    for i in range(3):
        for j in range(3):
            WS[i * C:(i + 1) * C, j * OUT:(j + 1) * OUT] = Wp[i, j].T
    WS[96, 0:OUT] = bias
    ones = np.ones((1, n * n))
    LT1 = L.T[PERM][:, PERM]
    LT2 = L2.T[PERM][:, PERM]
    return (G.astype(np.float16), WS.astype(np.float16),
            np.ascontiguousarray(LT1).astype(np.float16),
            np.ascontiguousarray(LT2).astype(np.float16),
            ones.astype(np.float16))


def _prep_in_maps(x, adj, weight, bias):
    x = np.asarray(x)
    G, WS, LT1, LT2, ONES = _host_inputs(adj, weight, bias)
    in_maps = []
    for b in range(B):
        xb = np.asarray(x[b], np.float16)
        xt = xb.transpose(2, 1, 0)[:, PERM, :]
        # x cols ordered (n2blk, c, r): col = blk*128 + c*4 + r, n2 = 4blk+r
        xd = xb.reshape(N, N // 4, 4, C).transpose(0, 1, 3, 2)
        in_maps.append({
            "x": np.ascontiguousarray(xd.reshape(N, N * C)),
            "xt": np.ascontiguousarray(xt.reshape(C, N * N)),
            "g": G, "ws": WS, "lt1": LT1, "lt2": LT2, "ones": ONES,
        })
    return in_maps


_PROGRAM = None


def kernel(x, adj, weight, bias):
    global _PROGRAM
    in_maps = _prep_in_maps(x, adj, weight, bias)
    if _PROGRAM is None:
        _PROGRAM = build_program()
    res = bass_utils.run_bass_kernel_spmd(_PROGRAM, in_maps,
                                          core_ids=list(range(B)))
    # device out is [n2_mem, n1, o] fp16 -> unpermute n2, transpose back
    out = np.empty((B, N, N, OUT), np.float32)
    for b in range(B):
        full = np.empty((N, N, OUT), np.float32)
        full[PERM] = res.results[b]["out"]
        out[b] = full.transpose(1, 0, 2)
    return out


# revision 15
# speedup vs baseline: 1.1159x; 1.1159x over previous
"""ChebConv2D (K1=K2=3) Trainium2 Bass kernel.

Data-parallel over batch (B=8) across 8 NeuronCores; per core the whole
per-batch computation runs on-chip.

Math (per batch, x: [N, N, C], N=200, C=32, OUT=64):
    out = U_0 + R_L(U_1) + R_{L^2}(U_2) + bias
    U_j = sum_i (A^i x) @ W'_ij^T      (Chebyshev folded into W' on host)

v5: no DRAM scratch. i=0 power comes from a host-transposed copy of x
DMA'd straight into TT; S1 computes only the L and L^2 powers and
scatters them into TT via SBUF->SBUF DMAs (4 chunks per DMA, 1600B
descriptors; x cols host-reordered c-major so no partition reorder is
needed). Output is stored [n2_mem, n1, o] fp16 with 5 blocks per store
DMA; host unpermutes/transposes/upcasts. Block loop is software-
pipelined (S3 of blk-1 between S2h1 of blk); DMAs spread over
sync/scalar/gpsimd queues.
"""

import numpy as np

import concourse.bass as bass
import concourse.mybir as mybir
from concourse import bacc
import concourse.tile as tile
from concourse import bass_utils

N = 200
C = 32
OUT = 64
B = 8
NC_HALF = 100
BLK = 8
NBLK = N // BLK
F32 = mybir.dt.float32
F16 = mybir.dt.float16
MIXN = 192
SUP = 4           # S1 chunks per scatter super-chunk
NSUP = 13         # ceil(50 / 4); last super has 2 chunks
OGRP = 5          # output blocks per store DMA
XCH = 10          # x chunks per load group


def build_program():
    nc = bacc.Bacc("TRN2")

    x_d = nc.dram_tensor("x", [N, N * C], F16, kind="ExternalInput")
    xt_d = nc.dram_tensor("xt", [C, N * N], F16, kind="ExternalInput")
    g_d = nc.dram_tensor("g", [N, 2 * N], F16, kind="ExternalInput")
    ws_d = nc.dram_tensor("ws", [C * 3 + 1, MIXN], F16, kind="ExternalInput")
    lt1_d = nc.dram_tensor("lt1", [N, N], F16, kind="ExternalInput")
    lt2_d = nc.dram_tensor("lt2", [N, N], F16, kind="ExternalInput")
    ones_d = nc.dram_tensor("ones", [1, N * N], F16, kind="ExternalInput")
    # out is [n2_mem, n1, o]; host unpermutes + transposes back
    out_d = nc.dram_tensor("out", [N, N, OUT], F16, kind="ExternalOutput")

    with tile.TileContext(nc) as tc:
        with (
            tc.tile_pool(name="const", bufs=1) as constp,
            tc.tile_pool(name="tt", bufs=1) as ttp,
            tc.tile_pool(name="u0", bufs=1) as u0p,
        ):
            g_t = []
            lt_t = {}
            for t in range(2):
                g = constp.tile([NC_HALF, 2 * N], F16, tag=f"g{t}")
                nc.sync.dma_start(g[:], g_d[t * NC_HALF:(t + 1) * NC_HALF, :])
                g_t.append(g)
            ws = constp.tile([C * 3 + 1, MIXN], F16, tag="ws")
            nc.sync.dma_start(ws[:], ws_d[:, :])
            for t in range(2):
                for j in (1, 2):
                    lt = constp.tile([NC_HALF, N], F16, tag=f"lt{j}{t}")
                    src = lt1_d if j == 1 else lt2_d
                    nc.sync.dma_start(lt[:], src[t * NC_HALF:(t + 1) * NC_HALF, :])
                    lt_t[(j, t)] = lt

            TT = ttp.tile([C * 3 + 1, N * N], F16, tag="TT")
            # i=0 power = x itself, transposed+permuted on host
            nc.gpsimd.dma_start(TT[0:C, :], xt_d[:, :])
            nc.gpsimd.dma_start(TT[96:97, :], ones_d[:, :])
            TT3 = TT[:].rearrange("p (a b) -> p a b", b=N)

            # U half 0 (mem rows 0:100) for all n1: [m, n1*192 + (j,o)]
            UC0 = u0p.tile([NC_HALF, N * MIXN], F16, tag="UC0")

            with (
                tc.tile_pool(name="xa", bufs=4) as xap,
                tc.tile_pool(name="sg", bufs=2) as sgp,
                tc.tile_pool(name="uc", bufs=4) as ucp,
                tc.tile_pool(name="ob", bufs=1) as obp,
                tc.tile_pool(name="psU", bufs=4, space="PSUM") as psup,
            ):
                psap_cm = tc.tile_pool(name="psA", bufs=3, space="PSUM")
                psap = psap_cm.__enter__()
                xt_big = [None, None]
                xq = {}

                def x_load(m):
                    eng = nc.sync if (m // XCH) % 2 == 0 else nc.scalar
                    for t in range(2):
                        xm = xap.tile([NC_HALF, XCH * 128], F16,
                                      tag=f"xm{t}", name=f"xm{t}_{m}")
                        eng.dma_start(
                            xm[:], x_d[t * NC_HALF:(t + 1) * NC_HALF,
                                       m * 128:(m + XCH) * 128])
                        xq[(t, m)] = xm

                def s1_super(s):
                    k0 = s * SUP
                    nk = min(SUP, 50 - k0)
                    # sc free layout: (i 2, k SUP, e N)
                    sc = sgp.tile([128, SUP * 2 * N], F16, tag="sc",
                                  name=f"sc_{s}")
                    sc4 = sc[:].rearrange("p (i k e) -> p i k e", i=2, e=N)
                    for k in range(nk):
                        m = k0 + k
                        if m % XCH == 0:
                            if (0, m) not in xq:
                                x_load(m)
                            for t in range(2):
                                xt_big[t] = xq[(t, m)]
                        if m % XCH == 2 and m + XCH - 2 < 50:
                            if (0, (m // XCH + 1) * XCH) not in xq:
                                x_load((m // XCH + 1) * XCH)
                        mm = m % XCH
                        psa = psap.tile([128, 2 * N], F32, tag="psa")
                        for t in range(2):
                            lhsT = xt_big[t][:, mm * 128:(mm + 1) * 128]
                            nc.tensor.matmul(psa[:], lhsT, g_t[t][:, :],
                                             start=(t == 0), stop=(t == 1))
                        dst = sc4[:, :, k, :]
                        src = psa[:].rearrange("p (i e) -> p i e", e=N)
                        if k % 2 == 0:
                            nc.vector.tensor_copy(dst, src)
                        else:
                            nc.scalar.copy(dst, src)
                    # scatter: sc[(c r), (i k e)] -> TT[(i c), mem 16s+4r+k, e]
                    eng = nc.sync if s % 2 == 0 else nc.scalar
                    for i in range(2):
                        src = sc[:, i * SUP * N:i * SUP * N + nk * N]
                        dst = TT3[(1 + i) * C:(2 + i) * C,
                                  4 * k0:4 * (k0 + nk), :]
                        eng.dma_start(dst, src)

                def s2h0_pair(p2):
                    psu = psup.tile([NC_HALF, 2 * MIXN], F32, tag="psu",
                                    name=f"psu0_{p2}")
                    for q in range(2):
                        n1 = p2 * 2 + q
                        lhsT = TT3[0:97, 0:NC_HALF, n1:n1 + 1]
                        nc.tensor.matmul(psu[:, q * MIXN:(q + 1) * MIXN],
                                         lhsT, ws[:], start=True, stop=True)
                    dst = UC0[:].rearrange("p (n f) -> p n f", f=MIXN)[
                        :, p2 * 2:p2 * 2 + 2, :]
                    psu3 = psu[:].rearrange("p (q f) -> p q f", f=MIXN)
                    if p2 % 2 == 0:
                        nc.vector.tensor_copy(dst, psu3)
                    else:
                        nc.scalar.copy(dst, psu3)

                # S1 supers 0..6 cover mem rows 0..111 (>= half 0), then
                # interleave remaining supers with S2 h=0
                for s in range(7):
                    s1_super(s)
                p2done = 0
                for s in range(7, NSUP):
                    s1_super(s)
                    target = (s - 6) * 14
                    while p2done < min(target, 100):
                        s2h0_pair(p2done)
                        p2done += 1
                while p2done < 100:
                    s2h0_pair(p2done)
                    p2done += 1
                psap_cm.__exit__(None, None, None)

                # ---- S2 h=1 + S3 per block, software-pipelined ----
                UC03 = UC0[:].rearrange("p (n f) -> p n f", f=MIXN)
                psop = ctx_psop = tc.tile_pool(name="psO", bufs=2, space="PSUM")
                psop = psop.__enter__()
                obig = {}
                uc1map = {}

                def s2h1_blk(blk):
                    uc1 = ucp.tile([NC_HALF, BLK * MIXN], F16, tag="uc1",
                                   name=f"uc1_{blk}")
                    for bi2 in range(BLK // 2):
                        psu = psup.tile([NC_HALF, 2 * MIXN], F32, tag="psu")
                        for q in range(2):
                            n1 = blk * BLK + bi2 * 2 + q
                            lhsT = TT3[0:97, NC_HALF:N, n1:n1 + 1]
                            nc.tensor.matmul(psu[:, q * MIXN:(q + 1) * MIXN],
                                             lhsT, ws[:], start=True, stop=True)
                        dst = uc1[:].rearrange("p (n f) -> p n f", f=MIXN)[
                            :, bi2 * 2:bi2 * 2 + 2, :]
                        psu3 = psu[:].rearrange("p (q f) -> p q f", f=MIXN)
                        if bi2 == 0:
                            nc.vector.tensor_copy(dst, psu3)
                        else:
                            nc.scalar.copy(dst, psu3)
                    uc1map[blk] = uc1[:].rearrange("p (n f) -> p n f", f=MIXN)

                def s3_blk(blk):
                    uc13 = uc1map.pop(blk)
                    gi = blk // OGRP
                    go = blk % OGRP
                    if go == 0:
                        for m2 in range(2):
                            obig[(gi, m2)] = obp.tile(
                                [NC_HALF, OGRP * BLK * OUT], F16, tag=f"ob{m2}",
                                name=f"ob{m2}_{gi}")
                    for m2 in range(2):
                        pso = psop.tile([NC_HALF, BLK * OUT], F32, tag="pso")
                        k = 0
                        for j in (1, 2):
                            for h in range(2):
                                lhsT = lt_t[(j, h)][:, m2 * NC_HALF:(m2 + 1) * NC_HALF]
                                u3 = UC03 if h == 0 else uc13
                                if h == 0:
                                    rhs = u3[:, blk * BLK:(blk + 1) * BLK,
                                             j * OUT:(j + 1) * OUT]
                                else:
                                    rhs = u3[:, :, j * OUT:(j + 1) * OUT]
                                nc.tensor.matmul(pso[:], lhsT, rhs,
                                                 start=(k == 0), stop=(k == 3))
                                k += 1
                        pso3 = pso[:].rearrange("p (n o) -> p n o", o=OUT)
                        u0s = (UC03 if m2 == 0 else uc13)
                        if m2 == 0:
                            u0 = u0s[:, blk * BLK:(blk + 1) * BLK, 0:OUT]
                        else:
                            u0 = u0s[:, :, 0:OUT]
                        ob3 = obig[(gi, m2)][:].rearrange(
                            "p (g n o) -> p g n o", g=OGRP, o=OUT)[:, go]
                        nc.vector.tensor_add(ob3, pso3, u0)
                        if go == OGRP - 1:
                            dst = out_d[m2 * NC_HALF:(m2 + 1) * NC_HALF,
                                        gi * OGRP * BLK:(gi + 1) * OGRP * BLK, :]
                            src = obig.pop((gi, m2))[:].rearrange(
                                "p (n o) -> p n o", o=OUT)
                            nc.gpsimd.dma_start(dst, src)

                s2h1_blk(0)
                for blk in range(1, NBLK):
                    s2h1_blk(blk)
                    s3_blk(blk - 1)
                s3_blk(NBLK - 1)
                ctx_psop.__exit__(None, None, None)
    nc.compile()
    return nc


def _perm():
    # TT node-dim memory order: within a full super (4 chunks of 4 rows),
    # mem row 16s+4r+k holds logical n2 16s+4k+r; tail super (2 chunks):
    # mem 192+2r+k holds logical 192+4k+r.
    P = np.zeros(N, np.int64)
    for s in range(12):
        base = 16 * s
        for a in range(4):
            for b in range(4):
                P[base + 4 * a + b] = base + 4 * b + a
    for r in range(4):
        for k in range(2):
            P[192 + 2 * r + k] = 192 + 4 * k + r
    return P


PERM = _perm()


def _host_inputs(adj, weight, bias):
    adj = np.asarray(adj, np.float64)
    weight = np.asarray(weight, np.float64)
    bias = np.asarray(bias, np.float64)
    n = adj.shape[0]
    A = adj * (1.0 - np.eye(n))
    d0 = A.sum(0) ** -0.5
    d1 = A.sum(1) ** -0.5
    d0[np.isinf(d0)] = 0.0
    d1[np.isinf(d1)] = 0.0
    L = d0[:, None] * A * d1[None, :]
    L2 = L @ L

    p = np.array([[1.0, 0, 0], [0, 1.0, 0], [-1.0, 0, 2.0]])
    W = weight.reshape(OUT, 3, 3, C)
    Wp = np.einsum("ai,bj,oabc->ijoc", p, p, W)

    G = np.concatenate([L, L2], axis=1)
    WS = np.zeros((3 * C + 1, MIXN))
    for i in range(3):
        for j in range(3):
            WS[i * C:(i + 1) * C, j * OUT:(j + 1) * OUT] = Wp[i, j].T
    WS[96, 0:OUT] = bias
    ones = np.ones((1, n * n))
    LT1 = L.T[PERM][:, PERM]
    LT2 = L2.T[PERM][:, PERM]
    return (G.astype(np.float16), WS.astype(np.float16),
            np.ascontiguousarray(LT1).astype(np.float16),
            np.ascontiguousarray(LT2).astype(np.float16),
            ones.astype(np.float16))


def _prep_in_maps(x, adj, weight, bias):
    x = np.asarray(x)
    G, WS, LT1, LT2, ONES = _host_inputs(adj, weight, bias)
    in_maps = []
    for b in range(B):
        xb = np.asarray(x[b], np.float16)
        xt = xb.transpose(2, 1, 0)[:, PERM, :]
        # x cols ordered (n2blk, c, r): col = blk*128 + c*4 + r, n2 = 4blk+r
        xd = xb.reshape(N, N // 4, 4, C).transpose(0, 1, 3, 2)
        in_maps.append({
            "x": np.ascontiguousarray(xd.reshape(N, N * C)),
            "xt": np.ascontiguousarray(xt.reshape(C, N * N)),
            "g": G, "ws": WS, "lt1": LT1, "lt2": LT2, "ones": ONES,
        })
    return in_maps


_PROGRAM = None


def kernel(x, adj, weight, bias):
    global _PROGRAM
    in_maps = _prep_in_maps(x, adj, weight, bias)
    if _PROGRAM is None:
        _PROGRAM = build_program()
    res = bass_utils.run_bass_kernel_spmd(_PROGRAM, in_maps,
                                          core_ids=list(range(B)))
    # device out is [n2_mem, n1, o] fp16 -> unpermute n2, transpose back
    out = np.empty((B, N, N, OUT), np.float32)
    for b in range(B):
        full = np.empty((N, N, OUT), np.float32)
        full[PERM] = res.results[b]["out"]
        out[b] = full.transpose(1, 0, 2)
    return out
